# revision 27
# baseline (speedup 1.0000x reference)
"""Bass/Trainium2 kernel for BiLinearLayer.

reference math (per batch b):
    att = relu(q1 @ U @ q2^T)            [T1, T2]
    w1  = softmax(att, axis=T1)          (column softmax)
    w2  = softmax(att, axis=T2)          (row softmax)
    q1_align = w1^T @ q1                 [T2, D]
    q2_align = w2 @ q2                   [T1, D]
returns (q1_align, q2_align), each [B, T, D] float32.

Sharding: data-parallel over batch B across 8 NeuronCores (8 batches/core),
U replicated.

Precision: both big matmuls (P = q1@U and att = P@q2^T) run single-pass
fp32r (products round to ~fp22). The peaked softmax amplifies the att
rounding error to ~8e-3 relative output error, inside the 2e-2 gate.
att is kept in full fp32 and transposed on TensorE in fp32 transpose mode
(rounding att before the max-subtraction would double the error). The
align matmuls run bf16 (weights are exp values in [0,1], values bf16).

Softmax weights are never transposed on the PE: both orientations are
recomputed directly from attr/attT as exp(att + bias) where
bias = -max - ln(sum) varies along the FREE dim. The bias columns are
packed [P, 8], PE-transposed once (tiny), row-broadcast across partitions
by GpSimd partition_broadcast, added on DVE, exponentiated on ACT straight
into bf16. Normalization is folded into the bias, so align outputs are
final and the PSUM->SBUF copy is a plain copy. Align outputs are produced
d-major ([D, T] per batch, lhsT = q-values, rhs = exp weights); the host
de-transposes.

All DMA'd tensors are host-blocked partition-major ([P, blocks, free]) so
every descriptor moves a 4-16KB contiguous line per partition. Batch 0's
P^T phase accumulates db-major across 4 concurrent PSUM banks per half so
the PE starts after the first 512KB U chunk + 256KB q1t chunk land.

Schedule: batches are software-pipelined. Batch i's bias transpose rides
between batch i's align halves... batch i's aligns run after batch i+1's
attT phase; its bias machinery (broadcast/add/exp) fills DVE/ACT/GpSimd
under batch i+1's matmuls. Output DMAs and the U preload ride the
scalar-engine HWDGE ring; input loads ride the sync ring.
"""

import sys

if "/opt/trn_rl_repo" not in sys.path:
    sys.path.insert(0, "/opt/trn_rl_repo")

from contextlib import ExitStack

import numpy as np

import concourse.bass as bass
import concourse.mybir as mybir
import concourse.tile as tile
from concourse import bacc
from concourse.masks import make_identity

F32 = mybir.dt.float32
F32R = mybir.dt.float32r
BF16 = mybir.dt.bfloat16
AF = mybir.ActivationFunctionType
AX = mybir.AxisListType
SUB = mybir.AluOpType.subtract
ADD = mybir.AluOpType.add

B, T, D = 64, 512, 1024
NCORES = 8
BL = B // NCORES  # batches per core
P = 128
TB = T // P  # 4 t/s blocks
DB = D // P  # 8 d/e blocks


def build_nc():
    nc = bacc.Bacc()
    # host-blocked layouts: partition dim first, 4-16KB lines per partition
    q1t = nc.dram_tensor("q1t", [BL, P, DB, T], F32R, kind="ExternalInput")
    q2t = nc.dram_tensor("q2t", [BL, P, DB, T], F32R, kind="ExternalInput")
    q1n = nc.dram_tensor("q1n", [BL, P, TB, D], BF16, kind="ExternalInput")
    q2n = nc.dram_tensor("q2n", [BL, P, TB, D], BF16, kind="ExternalInput")
    u = nc.dram_tensor("u", [P, DB, D], F32R, kind="ExternalInput")
    # outputs are d-major (transposed); host de-transposes
    o1 = nc.dram_tensor("o1", [BL, D, T], F32, kind="ExternalOutput")
    o2 = nc.dram_tensor("o2", [BL, D, T], F32, kind="ExternalOutput")

    with tile.TileContext(nc) as tc, ExitStack() as ctx:
        const = ctx.enter_context(tc.tile_pool(name="const", bufs=1))
        q_p = ctx.enter_context(tc.tile_pool(name="qt", bufs=4))
        qn_p = ctx.enter_context(tc.tile_pool(name="qn", bufs=4))
        pt_p = ctx.enter_context(tc.tile_pool(name="pt", bufs=1))
        att_p = ctx.enter_context(tc.tile_pool(name="att", bufs=2))
        e_p = ctx.enter_context(tc.tile_pool(name="e", bufs=4))
        b_p = ctx.enter_context(tc.tile_pool(name="b", bufs=2))
        st_p = ctx.enter_context(tc.tile_pool(name="st", bufs=2))
        out_p = ctx.enter_context(tc.tile_pool(name="out", bufs=3))
        ps_mm = ctx.enter_context(tc.tile_pool(name="ps_mm", bufs=4, space="PSUM"))
        ps_tr = ctx.enter_context(tc.tile_pool(name="ps_tr", bufs=3, space="PSUM"))

        ident_f32 = const.tile([P, P], F32)
        make_identity(nc, ident_f32[:])

        # U resident in fp32r, loaded per-db-chunk on the scalar HWDGE ring
        # (4KB lines); chunk 0 lands in ~1.5us so batch 0 starts early.
        u_sb = const.tile([P, DB, D], F32R)
        for db in range(DB):
            nc.scalar.dma_start(out=u_sb[:, db, :], in_=u[:, db, :])

        def load_n(i):
            """q1n/q2n for batch i's aligns; issued during batch i+1's
            matmul phase so the DMA hides under compute."""
            n1 = qn_p.tile([P, TB, D], BF16, tag="qn", name="n1")
            nc.sync.dma_start(out=n1[:], in_=q1n[i])
            n2 = qn_p.tile([P, TB, D], BF16, tag="qn", name="n2")
            nc.sync.dma_start(out=n2[:], in_=q2n[i])
            return n1, n2

        def pt_att_phase(i, prev_st=None):
            """Dense matmul phase of batch i + softmax stats."""
            t1 = q_p.tile([P, DB, T], F32R, tag="qt", name="t1")
            if i == 0:
                for db in range(DB):
                    nc.sync.dma_start(out=t1[:, db, :], in_=q1t[i][:, db, :])
            else:
                nc.sync.dma_start(out=t1[:], in_=q1t[i])

            # P^T[e,t] = sum_db U[db,e]^T q1t[db,t], single-pass fp32r
            pt = pt_p.tile([P, DB, T], F32R, tag="pt", name="pt")
            if i == 0:
                # db-major across 4 concurrent PSUM banks per half: the first
                # matmul only needs the first U + q1t chunks, not the full
                # 6MB preload.
                for half in range(2):
                    grps = [
                        ps_mm.tile([P, T], F32, tag="psmm", name="psmm")
                        for _ in range(4)
                    ]
                    for db in range(DB):
                        for k in range(4):
                            eb = half * 4 + k
                            nc.tensor.matmul(
                                grps[k][:],
                                u_sb[:, db, eb * P : (eb + 1) * P],
                                t1[:, db, :],
                                start=(db == 0),
                                stop=(db == DB - 1),
                            )
                    for k in range(4):
                        eb = half * 4 + k
                        if k % 2 == 0:
                            nc.vector.tensor_copy(pt[:, eb, :], grps[k][:])
                        else:
                            nc.scalar.copy(pt[:, eb, :], grps[k][:])
            else:
                for eb in range(DB):
                    ps = ps_mm.tile([P, T], F32, tag="psmm", name="psmm")
                    for db in range(DB):
                        nc.tensor.matmul(
                            ps[:],
                            u_sb[:, db, eb * P : (eb + 1) * P],
                            t1[:, db, :],
                            start=(db == 0),
                            stop=(db == DB - 1),
                        )
                    nc.vector.tensor_copy(pt[:, eb, :], ps[:])

            t2 = q_p.tile([P, DB, T], F32R, tag="qt", name="t2")
            nc.sync.dma_start(out=t2[:], in_=q2t[i])
            if prev_st is not None:
                prev_st["n1"], prev_st["n2"] = load_n(i - 1)
            st = {}
            if i == BL - 1:
                st["n1"], st["n2"] = load_n(i)

            # stat columns: nm = -max, sm = sum(exp(att-max)), per direction
            nm2c = st_p.tile([P, TB], F32, tag="nm2", name="nm2c")
            sm2c = st_p.tile([P, TB], F32, tag="sm2", name="sm2c")
            nm1c = st_p.tile([P, TB], F32, tag="nm1", name="nm1c")
            sm1c = st_p.tile([P, TB], F32, tag="sm1", name="sm1c")
            dead = e_p.tile([P, TB, T], BF16, tag="ed", name="dead", bufs=1)

            # att[t,s] = sum_eb P^T[eb,t]^T q2t[eb,:], single-pass fp32r; relu
            # + rowmax/rowsum per block as soon as its relu lands
            attr = att_p.tile([P, TB, T], F32, tag="att", name="attr")
            for tb in range(TB):
                ps = ps_mm.tile([P, T], F32, tag="psmm", name="psmm")
                for eb in range(DB):
                    nc.tensor.matmul(
                        ps[:],
                        pt[:, eb, tb * P : (tb + 1) * P],
                        t2[:, eb, :],
                        start=(eb == 0),
                        stop=(eb == DB - 1),
                    )
                nc.scalar.activation(attr[:, tb, :], ps[:], AF.Relu)
                nc.vector.reduce_max(
                    out=nm2c[:, tb : tb + 1], in_=attr[:, tb, :], axis=AX.X,
                    op=mybir.AluOpType.max, negate=True,
                )
                nc.scalar.activation(
                    dead[:, tb, :], attr[:, tb, :], AF.Exp,
                    bias=nm2c[:, tb : tb + 1], accum_out=sm2c[:, tb : tb + 1],
                )

            # attT[s,t] via fp32 PE transpose (full precision)
            attT = att_p.tile([P, TB, T], F32, tag="att", name="attT")
            for sb in range(TB):
                ps = ps_tr.tile([P, T], F32, tag="pstr", name="pstr")
                for tb in range(TB):
                    nc.tensor.transpose(
                        ps[:, tb * P : (tb + 1) * P],
                        attr[:, tb, sb * P : (sb + 1) * P],
                        ident_f32[:],
                    )
                nc.scalar.copy(attT[:, sb, :], ps[:])

            # colmax/colsum (needs attT); runs on DVE/ACT under the next
            # batch's matmuls.
            for sb in range(TB):
                nc.vector.reduce_max(
                    out=nm1c[:, sb : sb + 1], in_=attT[:, sb, :], axis=AX.X,
                    op=mybir.AluOpType.max, negate=True,
                )
                nc.scalar.activation(
                    dead[:, sb, :], attT[:, sb, :], AF.Exp,
                    bias=nm1c[:, sb : sb + 1], accum_out=sm1c[:, sb : sb + 1],
                )

            # packed bias columns bc = [-rm - ln s2 | -cm - ln s1]
            lnc = st_p.tile([P, 2 * TB], F32, tag="ln", name="lnc")
            nc.scalar.activation(lnc[:, 0:TB], sm2c[:], AF.Ln)
            nc.scalar.activation(lnc[:, TB : 2 * TB], sm1c[:], AF.Ln)
            bc = st_p.tile([P, 2 * TB], F32, tag="bc", name="bc")
            nc.vector.tensor_tensor(
                out=bc[:, 0:TB], in0=nm2c[:], in1=lnc[:, 0:TB], op=SUB
            )
            nc.vector.tensor_tensor(
                out=bc[:, TB : 2 * TB], in0=nm1c[:], in1=lnc[:, TB : 2 * TB],
                op=SUB,
            )
            st.update(attr=attr, attT=attT, bc=bc)
            return st

        def bias_exp_phase(i, st):
            """Transpose packed bias cols to rows (tiny PE op), broadcast
            across partitions on GpSimd, then recompute both exp
            orientations straight into bf16 (DVE add + ACT exp)."""
            ps = ps_tr.tile([P, P], F32, tag="bctr", name="bctr", bufs=1)
            nc.tensor.transpose(ps[0 : 2 * TB, :], st["bc"][:], ident_f32[:])
            bcr = st_p.tile([2 * TB, P], F32, tag="bcr", name="bcr")
            nc.vector.tensor_copy(bcr[:], ps[0 : 2 * TB, :])
            # flatten rows onto partition 0 (broadcast src must be part 0)
            brow = st_p.tile([1, 2 * TB, P], F32, tag="brow", name="brow", bufs=1)
            nc.scalar.dma_start(out=brow[:], in_=bcr[:])

            b2b = b_p.tile([P, T], F32, tag="bb", name="b2b")
            b1b = b_p.tile([P, T], F32, tag="bb", name="b1b")
            for tb in range(TB):
                nc.gpsimd.partition_broadcast(
                    b2b[:, tb * P : (tb + 1) * P], brow[0:1, tb, :]
                )
            for sb in range(TB):
                nc.gpsimd.partition_broadcast(
                    b1b[:, sb * P : (sb + 1) * P], brow[0:1, TB + sb, :]
                )

            e2T = e_p.tile([P, TB, T], BF16, tag="et", name="e2T")
            e1 = e_p.tile([P, TB, T], BF16, tag="et", name="e1")
            b2f = b2b
            b1f = b1b
            sub = att_p.tile([P, TB, T], F32, tag="sub", name="sub", bufs=1)
            for sb in range(TB):
                nc.vector.tensor_tensor(
                    out=sub[:, sb, :], in0=st["attT"][:, sb, :], in1=b2f, op=ADD
                )
                nc.scalar.activation(e2T[:, sb, :], sub[:, sb, :], AF.Exp)
            for tb in range(TB):
                nc.vector.tensor_tensor(
                    out=sub[:, tb, :], in0=st["attr"][:, tb, :], in1=b1f, op=ADD
                )
                nc.scalar.activation(e1[:, tb, :], sub[:, tb, :], AF.Exp)
            st["e2T"], st["e1"] = e2T, e1

        def aligns_phase(i, st, tail=False, mid=None):
            e2T, e1, n1, n2 = st["e2T"], st["e1"], st["n1"], st["n2"]
            # q2_alignT[d,t] = sum_sb n2[sb,d-blk]^T @ E2T[sb,:] (normalized)
            for db in range(DB):
                ps = ps_mm.tile([P, T], F32, tag="psmm", name="psmm")
                for sb in range(TB):
                    nc.tensor.matmul(
                        ps[:],
                        n2[:, sb, db * P : (db + 1) * P],
                        e2T[:, sb, :],
                        start=(sb == 0),
                        stop=(sb == TB - 1),
                    )
                ob = out_p.tile([P, T], F32, tag="out", name="ob2")
                if db % 2 == 0:
                    nc.vector.tensor_copy(ob[:], ps[:])
                else:
                    nc.scalar.copy(ob[:], ps[:])
                (nc.sync if tail else nc.scalar).dma_start(
                    out=o2[i, db * P : (db + 1) * P, :], in_=ob[:]
                )
            if mid is not None:
                mid()
            # q1_alignT[d,s] = sum_tb n1[tb,d-blk]^T @ E1[tb,:]
            for db in range(DB):
                ps = ps_mm.tile([P, T], F32, tag="psmm", name="psmm")
                for tb in range(TB):
                    nc.tensor.matmul(
                        ps[:],
                        n1[:, tb, db * P : (db + 1) * P],
                        e1[:, tb, :],
                        start=(tb == 0),
                        stop=(tb == TB - 1),
                    )
                ob = out_p.tile([P, T], F32, tag="out", name="ob1")
                if db % 2 == 0:
                    nc.vector.tensor_copy(ob[:], ps[:])
                else:
                    nc.scalar.copy(ob[:], ps[:])
                (nc.sync if tail else nc.scalar).dma_start(
                    out=o1[i, db * P : (db + 1) * P, :], in_=ob[:]
                )

        states = {}
        for i in range(BL):
            st = pt_att_phase(i, prev_st=states.get(i - 1))
            states[i] = st
            if i == 0:
                bias_exp_phase(0, st)
            else:
                aligns_phase(
                    i - 1, states[i - 1],
                    mid=(lambda ii=i: bias_exp_phase(ii, states[ii])),
                )
        aligns_phase(BL - 1, states[BL - 1], tail=True)

    nc.compile()
    return nc


def _pblock(x, blk):
    """[..., blk*P, F] -> [..., P, blk, F] partition-major blocking."""
    s = x.shape
    y = x.reshape(s[:-2] + (blk, P, s[-1]))
    y = np.moveaxis(y, -3, -2)
    return np.ascontiguousarray(y)


def prep_inputs(q1, q2, U):
    """Host-side layout/precision prep shared by kernel() and test harness."""
    import ml_dtypes

    q1 = np.ascontiguousarray(q1, dtype=np.float32)
    q2 = np.ascontiguousarray(q2, dtype=np.float32)
    U = np.ascontiguousarray(U, dtype=np.float32)
    q1t = q1.transpose(0, 2, 1)
    q2t = q2.transpose(0, 2, 1)
    bf = lambda x: x.astype(ml_dtypes.bfloat16)
    return {
        "q1t": _pblock(q1t, DB), "q2t": _pblock(q2t, DB),
        "q1n": bf(_pblock(q1, TB)), "q2n": bf(_pblock(q2, TB)),
        "u": _pblock(U, DB),
    }


_NC_CACHE = None


def _get_nc():
    global _NC_CACHE
    if _NC_CACHE is None:
        _NC_CACHE = build_nc()
    return _NC_CACHE


def kernel(q1: np.ndarray, q2: np.ndarray, U: np.ndarray):
    from concourse import bass_utils

    nc = _get_nc()
    full = prep_inputs(q1, q2, U)
    in_maps = []
    for c in range(NCORES):
        s = slice(c * BL, (c + 1) * BL)
        in_maps.append(
            {k: (v[s] if v.shape[0] == B else v) for k, v in full.items()}
        )
    res = bass_utils.run_bass_kernel_spmd(nc, in_maps, list(range(NCORES)))
    o1t = np.concatenate([res.results[c]["o1"] for c in range(NCORES)], axis=0)
    o2t = np.concatenate([res.results[c]["o2"] for c in range(NCORES)], axis=0)
    o1 = np.ascontiguousarray(o1t.transpose(0, 2, 1))
    o2 = np.ascontiguousarray(o2t.transpose(0, 2, 1))
    return (o1, o2)
